# revision 1
# baseline (speedup 1.0000x reference)
"""ComplexGAT Trainium2 kernel: builder + host preprocessing.

Sharding: dst-node blocks across 8 cores (SPMD, one program). Per layer:
  GEMM (W augmented with [a_src|a_dst] cols -> es/ed free) -> node-major table
  rows [xl | es | pad] -> AllGather -> per dst-block: dma_gather the block's
  edge-chunk window (768B rows, es rides), ed via S01^T matmuls (fp8 host
  patterns), w = exp(lrelu(es+ed)) on DVE+ACT, rhs = [G*w | w], scatter via
  fp8 S01 matmuls accumulating [u | den] in PSUM, normalize u/den.
  BN: mask-matmul stats + AllReduce + scale/shift broadcast via ones-matmul,
  relu; PE transpose-back for next GEMM lhsT. Final: pool matmul + AllReduce
  + linear.
Boundary chunks shared by two blocks are re-gathered; foreign edges get
garbage w but zero S01 columns, so they contribute nothing.
"""
import sys
import numpy as np
import ml_dtypes

sys.path.insert(0, "/opt/trn_rl_repo")
import concourse.bass as bass
import concourse.bacc as bacc
import concourse.mybir as mybir
import concourse.tile as tile
from concourse.masks import make_identity

P = 128
bf16 = mybir.dt.bfloat16
fp8 = mybir.dt.float8e4
f32 = mybir.dt.float32
NEG_SLOPE = 0.2
BN_EPS = 1e-5


class Cfg:
    def __init__(self, N=50000, E=800000, G=64, IN=512, HID=128, HEADS=2,
                 OUT=64, ncores=8):
        self.N, self.E, self.G, self.IN, self.HID, self.HEADS, self.OUT = \
            N, E, G, IN, HID, HEADS, OUT
        self.ncores = ncores
        self.npc = N // ncores
        assert self.npc * ncores == N
        self.nblk = (self.npc + P - 1) // P
        self.npc_pad = self.nblk * P
        self.Npad = self.npc_pad * ncores
        self.HC = HID * HEADS
        self.TW = 384 if HEADS == 2 else 256   # table row elems (bytes%256==0)


def host_prep(cfg, edge_index, batch):
    N, ncores = cfg.N, cfg.ncores
    src = np.asarray(edge_index[0], np.int64)
    dst = np.asarray(edge_index[1], np.int64)
    loops = np.arange(N, dtype=np.int64)
    src = np.concatenate([src, loops])
    dst = np.concatenate([dst, loops])
    half = cfg.Npad // 2

    def pad_id(n):
        return (n // cfg.npc) * cfg.npc_pad + (n % cfg.npc)

    src_p, dst_p = pad_id(src), pad_id(dst)
    core_of = dst // cfg.npc

    # per (core, li, block) edge groups
    groups_e = {}
    kmax = np.zeros(2, np.int64)
    for c in range(ncores):
        m = core_of == c
        s_c, d_c = src_p[m], dst_p[m]
        base = c * cfg.npc_pad
        for li in range(2):
            hm = (s_c >= half) == (li == 1)
            s_l, d_l = s_c[hm], d_c[hm]
            blk = (d_l - base) // P
            for b in range(cfg.nblk):
                bm = blk == b
                groups_e[(c, li, b)] = (s_l[bm], d_l[bm])
                kmax[li] = max(kmax[li], (bm.sum() + P - 1) // P)
    K = [int(kmax[0]), int(kmax[1])]
    Cn = [K[li] * cfg.nblk for li in range(2)]
    win = np.zeros((2, cfg.nblk, 2), np.int64)
    for li in range(2):
        for b in range(cfg.nblk):
            win[li, b] = (K[li] * b, K[li] * (b + 1))
    wmax = max(K)
    nch = [Cn[0], Cn[1]]

    per_core = []
    for c in range(ncores):
        base = c * cfg.npc_pad
        d = {}
        for li in range(2):
            Cl = Cn[li]
            idxs = np.zeros(Cl * P, np.int64)
            dloc = np.full(Cl * P, -1, np.int64)
            for b in range(cfg.nblk):
                s_b, d_b = groups_e[(c, li, b)]
                e0 = K[li] * b * P
                idxs[e0:e0 + s_b.size] = s_b - li * half
                dloc[e0:e0 + s_b.size] = d_b - base
            assert idxs.min() >= 0 and (idxs.max() < half or Cl == 0)
            iw = np.zeros((16, Cl * P // 16), np.int16)
            ar = np.arange(Cl * P)
            iw[ar % 16, ar // 16] = idxs
            d[f"idx{li}"] = np.tile(iw, (8, 1))
            S = np.zeros((max(nch[li], 1), P, P), np.float32)
            ST = np.zeros((max(nch[li], 1), P, P), np.float32)
            rows = np.arange(P)
            for b in range(cfg.nblk):
                for j in range(K[li]):
                    ch = K[li] * b + j
                    dl = dloc[ch * P:(ch + 1) * P]
                    ok = (dl >= b * P) & (dl < (b + 1) * P)
                    S[ch, rows[ok], dl[ok] - b * P] = 1.0
                    ST[ch] = S[ch].T
            d[f"s01_{li}"] = np.ascontiguousarray(
                S.transpose(1, 0, 2)).astype(ml_dtypes.float8_e4m3)
            d[f"s01t_{li}"] = np.ascontiguousarray(
                ST.transpose(1, 0, 2)).astype(ml_dtypes.float8_e4m3)
        per_core.append(d)

    Gn = cfg.G
    batch = np.asarray(batch, np.int64)
    cnt = np.bincount(batch, minlength=Gn).astype(np.float64)
    inv_cnt = (1.0 / np.maximum(cnt, 1.0)).astype(np.float32)[:, None]
    for c in range(ncores):
        p01 = np.zeros((cfg.nblk, P, Gn), np.float32)
        mask = np.zeros((P, cfg.nblk), np.float32)
        for b in range(cfg.nblk):
            for p_ in range(P):
                n_loc = b * P + p_
                if n_loc < cfg.npc:
                    p01[b, p_, batch[c * cfg.npc + n_loc]] = 1.0
                    mask[p_, b] = 1.0
        per_core[c]["p01"] = p01.astype(ml_dtypes.bfloat16)
        per_core[c]["mask"] = mask.astype(ml_dtypes.bfloat16)
        per_core[c]["inv_cnt"] = inv_cnt
    meta = {"Cn": Cn, "win": win, "wmax": wmax, "nch": nch}
    return per_core, meta


def build_weights(W1, as1, ad1, W2, as2, ad2, W3, as3, ad3, Wlin, b3, blin,
                  g1, be1, g2, be2):
    def aug(W, a_s, a_d):
        H, C = np.asarray(a_s).shape
        A = np.zeros((W.shape[1], 2 * H), np.float32)
        for h in range(H):
            A[h * C:(h + 1) * C, h] = a_s[h]
            A[h * C:(h + 1) * C, H + h] = a_d[h]
        return np.concatenate([np.asarray(W, np.float32),
                               np.asarray(W, np.float32) @ A], 1)
    return {
        "w1a": aug(W1, as1, ad1).astype(ml_dtypes.bfloat16),
        "w2a": aug(W2, as2, ad2).astype(ml_dtypes.bfloat16),
        "w3a": aug(W3, as3, ad3).astype(ml_dtypes.bfloat16),
        "wlin": np.asarray(Wlin).astype(ml_dtypes.bfloat16),
        "bn1": np.stack([g1, be1]).astype(np.float32),
        "bn2": np.stack([g2, be2]).astype(np.float32),
        "b3row": np.asarray(b3)[None, :].astype(np.float32),
        "blinrow": np.asarray(blin)[None, :].astype(np.float32),
    }


def build_kernel(cfg, meta):
    nc = bacc.Bacc("TRN2", target_bir_lowering=False, debug=False,
                   num_devices=cfg.ncores, enable_asserts=False)
    HC, HID, HEADS = cfg.HC, cfg.HID, cfg.HEADS
    nblk = cfg.nblk
    Cn, win, wmax, nch = meta["Cn"], meta["win"], meta["wmax"], meta["nch"]
    NPP = cfg.npc_pad
    TW = cfg.TW
    half_rows = cfg.Npad // 2
    KMAX = max(cfg.IN, HC) // P

    xT = nc.dram_tensor("xT", [cfg.IN, NPP], bf16, kind="ExternalInput")
    w1a = nc.dram_tensor("w1a", [cfg.IN, HC + 2 * HEADS], bf16, kind="ExternalInput")
    w2a = nc.dram_tensor("w2a", [HC, HC + 2 * HEADS], bf16, kind="ExternalInput")
    w3a = nc.dram_tensor("w3a", [HC, HID + 2], bf16, kind="ExternalInput")
    wlin = nc.dram_tensor("wlin", [HID, cfg.OUT], bf16, kind="ExternalInput")
    bn1 = nc.dram_tensor("bn1", [2, HC], f32, kind="ExternalInput")
    bn2 = nc.dram_tensor("bn2", [2, HC], f32, kind="ExternalInput")
    b3row = nc.dram_tensor("b3row", [1, HID], f32, kind="ExternalInput")
    blinrow = nc.dram_tensor("blinrow", [1, cfg.OUT], f32, kind="ExternalInput")
    idx_t = [nc.dram_tensor(f"idx{li}", [P, Cn[li] * P // 16], mybir.dt.int16,
                            kind="ExternalInput") for li in range(2)]
    s01_t = [nc.dram_tensor(f"s01_{li}", [P, max(nch[li], 1), P], fp8,
                            kind="ExternalInput") for li in range(2)]
    s01t_t = [nc.dram_tensor(f"s01t_{li}", [P, max(nch[li], 1), P], fp8,
                             kind="ExternalInput") for li in range(2)]
    p01 = nc.dram_tensor("p01", [nblk, P, cfg.G], bf16, kind="ExternalInput")
    maskv = nc.dram_tensor("mask", [P, nblk], bf16, kind="ExternalInput")
    inv_cnt = nc.dram_tensor("inv_cnt", [cfg.G, 1], f32, kind="ExternalInput")
    out_t = nc.dram_tensor("out", [cfg.G, cfg.OUT], f32, kind="ExternalOutput")

    tab_loc = nc.dram_tensor("tab_loc", [NPP, TW], bf16)
    shr = "Shared" if cfg.ncores > 4 else "Local"
    tab_full = nc.dram_tensor("tab_full", [cfg.Npad, TW], bf16, addr_space=shr)
    stats_loc = nc.dram_tensor("stats_loc", [1, 2 * HC], f32)
    stats_full = nc.dram_tensor("stats_full", [1, 2 * HC], f32, addr_space=shr)
    pool_loc = nc.dram_tensor("pool_loc", [cfg.G, HID], f32)
    pool_full = nc.dram_tensor("pool_full", [cfg.G, HID], f32, addr_space=shr)
    groups = [list(range(cfg.ncores))]

    with tile.TileContext(nc) as tc:
        with tc.tile_pool(name="con", bufs=1) as con, \
             tc.tile_pool(name="sb", bufs=2) as sb, \
             tc.tile_pool(name="blk", bufs=2) as bp, \
             tc.tile_pool(name="ps", bufs=2, space="PSUM") as ps, \
             tc.tile_pool(name="ps2", bufs=2, space="PSUM") as ps2:

            hT = con.tile([P, HC // P, NPP], bf16, tag="hT")
            idx_sb = []
            for li in range(2):
                t_idx = con.tile([P, Cn[li] * P // 16], mybir.dt.int16,
                                 tag=f"idx{li}")
                idx_sb.append(t_idx)
            for li in range(2):
                nc.sync.dma_start(idx_sb[li][:], idx_t[li][:])
            mask_sb = con.tile([P, nblk], bf16, tag="mask")
            nc.sync.dma_start(mask_sb[:], maskv[:])
            ones_sb = con.tile([1, P], bf16, tag="ones")
            nc.gpsimd.memset(ones_sb[:], 1.0)
            ident = con.tile([P, P], bf16, tag="ident")
            make_identity(nc, ident[:])
            unorm = con.tile([P, nblk, HC], bf16, tag="unorm")
            edloc = con.tile([P, nblk, 2 * HEADS], bf16, tag="edloc")

            for layer in range(3):
                K_in = cfg.IN if layer == 0 else HC
                M = HC if layer < 2 else HID
                H = HEADS if layer < 2 else 1
                Wt = (w1a, w2a, w3a)[layer]
                wcols = M + 2 * H
                # ---- GEMM + table stage ----
                stage_b = con.tile([P, TW], bf16, tag="stage_b")
                nc.gpsimd.memset(stage_b[:], 0.0)
                w_sb = sb.tile([P, KMAX, HC + 2 * HEADS], bf16, tag="wsb")
                nc.sync.dma_start(w_sb[:, 0:K_in // P, 0:wcols],
                                  Wt[:].rearrange("(k p) m -> p k m", p=P))
                for b in range(nblk):
                    if layer == 0:
                        lhs_b = sb.tile([P, K_in // P, P], bf16, tag="lhsb")
                        nc.sync.dma_start(
                            lhs_b[:],
                            xT[:].rearrange("(k p) n -> p k n", p=P)
                            [:, :, b * P:(b + 1) * P])
                    g_ps = ps.tile([P, HC + 2 * HEADS], f32, tag="gemm")
                    for k in range(K_in // P):
                        lhsT_ap = (lhs_b[:, k, :] if layer == 0
                                   else hT[:, k, b * P:(b + 1) * P])
                        nc.tensor.matmul(g_ps[:, 0:wcols], lhsT_ap,
                                         w_sb[:, k, 0:wcols],
                                         start=(k == 0), stop=(k == K_in // P - 1))
                    nc.vector.tensor_copy(stage_b[:, 0:M + H], g_ps[:, 0:M + H])
                    nc.scalar.copy(edloc[:, b, 0:2 * H], g_ps[:, M:M + 2 * H])
                    nc.sync.dma_start(tab_loc[b * P:(b + 1) * P, :], stage_b[:])
                nc.gpsimd.collective_compute(
                    "AllGather", mybir.AluOpType.bypass, replica_groups=groups,
                    ins=[tab_loc[:].opt()], outs=[tab_full[:].opt()])

                # ---- per dst-block ----
                sptr = [0, 0]
                for b in range(nblk):
                    wins = [(int(win[li, b, 0]), int(win[li, b, 1]))
                            for li in range(2)]
                    Wb = [c1 - c0 for (c0, c1) in wins]
                    o_ps = ps.tile([P, HC + HEADS], f32, tag="ops")
                    first = True
                    for li in range(2):
                        c0, c1 = wins[li]
                        if Wb[li] == 0:
                            continue
                        G_t = bp.tile([P, wmax, TW], bf16, tag=f"G{li}")
                        nc.gpsimd.dma_gather(
                            out_ap=G_t[:, 0:Wb[li], :],
                            in_ap=tab_full[li * half_rows:(li + 1) * half_rows, :],
                            idxs_ap=idx_sb[li][:, c0 * 8:c1 * 8],
                            num_idxs=Wb[li] * P, num_idxs_reg=Wb[li] * P,
                            elem_size=TW, single_packet=False)
                        # ed matmuls (one per chunk) -> psum
                        st_sb = bp.tile([P, wmax, P], fp8, tag=f"st{li}")
                        nc.sync.dma_start(
                            st_sb[:, 0:Wb[li], :],
                            s01t_t[li][:, sptr[li]:sptr[li] + Wb[li], :])
                        ed_ps = ps2.tile([P, wmax, H], f32, tag="edps")
                        for j in range(Wb[li]):
                            nc.tensor.matmul(ed_ps[:, j, :], st_sb[:, j, :],
                                             edloc[:, b, H:2 * H],
                                             start=True, stop=True)
                        # z = es + ed ; lrelu ; w = exp
                        z_t = bp.tile([P, wmax, 2 * H], f32, tag=f"z{li}")
                        nc.vector.tensor_tensor(
                            out=z_t[:, 0:Wb[li], 0:H],
                            in0=G_t[:, 0:Wb[li], M:M + H],
                            in1=ed_ps[:, 0:Wb[li], :], op=mybir.AluOpType.add)
                        nc.vector.tensor_scalar(
                            out=z_t[:, 0:Wb[li], H:2 * H],
                            in0=z_t[:, 0:Wb[li], 0:H], scalar1=NEG_SLOPE,
                            scalar2=None, op0=mybir.AluOpType.mult)
                        nc.vector.tensor_tensor(
                            out=z_t[:, 0:Wb[li], 0:H], in0=z_t[:, 0:Wb[li], 0:H],
                            in1=z_t[:, 0:Wb[li], H:2 * H], op=mybir.AluOpType.max)
                        rhs_t = bp.tile([P, wmax, HC + HEADS], bf16, tag=f"rhs{li}")
                        nc.scalar.activation(rhs_t[:, 0:Wb[li], M:M + H],
                                             z_t[:, 0:Wb[li], 0:H],
                                             mybir.ActivationFunctionType.Exp)
                        nc.vector.tensor_tensor(
                            out=rhs_t[:, 0:Wb[li], 0:M].rearrange(
                                "p c (h f) -> p c h f", h=H),
                            in0=G_t[:, 0:Wb[li], 0:M].rearrange(
                                "p c (h f) -> p c h f", h=H),
                            in1=rhs_t[:, 0:Wb[li], M:M + H].to_broadcast(
                                [P, Wb[li], H, M // H]),
                            op=mybir.AluOpType.mult)
                        # scatter matmuls
                        s_sb = bp.tile([P, wmax, P], fp8, tag=f"s{li}")
                        nc.sync.dma_start(
                            s_sb[:, 0:Wb[li], :],
                            s01_t[li][:, sptr[li]:sptr[li] + Wb[li], :])
                        sptr[li] += Wb[li]
                        for j in range(Wb[li]):
                            last = (li == (1 if Wb[1] else 0)) and (j == Wb[li] - 1)
                            nc.tensor.matmul(o_ps[:, 0:M + H], s_sb[:, j, :],
                                             rhs_t[:, j, 0:M + H],
                                             start=first, stop=last)
                            first = False
                    # normalize
                    rec = sb.tile([P, HEADS], f32, tag="rec")
                    nc.vector.tensor_scalar(out=rec[:, 0:H], in0=o_ps[:, M:M + H],
                                            scalar1=1e-30, scalar2=None,
                                            op0=mybir.AluOpType.add)
                    nc.vector.reciprocal(rec[:, 0:H], rec[:, 0:H])
                    nc.vector.tensor_tensor(
                        out=unorm[:, b, 0:M].rearrange("p (h f) -> p h f", h=H),
                        in0=o_ps[:, 0:M].rearrange("p (h f) -> p h f", h=H),
                        in1=rec[:, 0:H].to_broadcast([P, H, M // H]),
                        op=mybir.AluOpType.mult)

                if layer < 2:
                    # ---- BN + relu + transpose-back ----
                    st_ps = ps.tile([1, 2, HC], f32, tag="gemm")
                    for b in range(nblk):
                        nc.tensor.matmul(st_ps[:, 0, :], mask_sb[:, b:b + 1],
                                         unorm[:, b, :], start=(b == 0),
                                         stop=(b == nblk - 1))
                    for b in range(nblk):
                        sq_b = sb.tile([P, HC], bf16, tag="sq_b")
                        nc.scalar.activation(sq_b[:], unorm[:, b, :],
                                             mybir.ActivationFunctionType.Square)
                        nc.tensor.matmul(st_ps[:, 1, :], mask_sb[:, b:b + 1],
                                         sq_b[:], start=(b == 0),
                                         stop=(b == nblk - 1))
                    st_sb2 = sb.tile([1, 2, HC], f32, tag="stsb")
                    nc.vector.tensor_copy(st_sb2[:], st_ps[:])
                    nc.sync.dma_start(stats_loc[:],
                                      st_sb2[:].rearrange("a b c -> a (b c)"))
                    nc.gpsimd.collective_compute(
                        "AllReduce", mybir.AluOpType.add, replica_groups=groups,
                        ins=[stats_loc[:].opt()], outs=[stats_full[:].opt()])
                    st2 = sb.tile([1, 2, HC], f32, tag="st2")
                    nc.sync.dma_start(st2[:].rearrange("a b c -> a (b c)"),
                                      stats_full[:])
                    bnp = sb.tile([1, 2, HC], f32, tag="bnp")
                    nc.sync.dma_start(bnp[:].rearrange("a b c -> a (b c)"),
                                      (bn1 if layer == 0 else bn2)[:].rearrange(
                                          "b c -> (b c)").rearrange(
                                          "(o k) -> o k", o=1))
                    tmp = sb.tile([1, 2, HC], f32, tag="tmp")
                    sc = sb.tile([1, 2, HC], f32, tag="sc")
                    # tmp = [mu, Ex2]
                    nc.vector.tensor_scalar(out=tmp[:], in0=st2[:],
                                            scalar1=1.0 / cfg.N, scalar2=None,
                                            op0=mybir.AluOpType.mult)
                    # sc[0] = var = Ex2 - mu^2
                    nc.vector.tensor_tensor(out=sc[:, 0, :], in0=tmp[:, 0, :],
                                            in1=tmp[:, 0, :], op=mybir.AluOpType.mult)
                    nc.vector.tensor_tensor(out=sc[:, 0, :], in0=tmp[:, 1, :],
                                            in1=sc[:, 0, :],
                                            op=mybir.AluOpType.subtract)
                    nc.vector.tensor_scalar(out=sc[:, 0, :], in0=sc[:, 0, :],
                                            scalar1=BN_EPS, scalar2=None,
                                            op0=mybir.AluOpType.add)
                    nc.scalar.activation(sc[:, 1, :], sc[:, 0, :],
                                         mybir.ActivationFunctionType.Sqrt)
                    nc.vector.reciprocal(sc[:, 1, :], sc[:, 1, :])
                    # sc[0] = scale = g * rsqrt ; sc[1] = shift = beta - mu*scale
                    nc.vector.tensor_tensor(out=sc[:, 0, :], in0=bnp[:, 0, :],
                                            in1=sc[:, 1, :], op=mybir.AluOpType.mult)
                    nc.vector.tensor_tensor(out=tmp[:, 0, :], in0=tmp[:, 0, :],
                                            in1=sc[:, 0, :], op=mybir.AluOpType.mult)
                    nc.vector.tensor_tensor(out=sc[:, 1, :], in0=bnp[:, 1, :],
                                            in1=tmp[:, 0, :],
                                            op=mybir.AluOpType.subtract)
                    scb = sb.tile([1, 2, HC], bf16, tag="scb")
                    nc.vector.tensor_copy(scb[:], sc[:])
                    bc_ps = ps.tile([P, 2 * HC], f32, tag="gemm")
                    nc.tensor.matmul(bc_ps[:], ones_sb[:],
                                     scb[:].rearrange("a b c -> a (b c)"),
                                     start=True, stop=True)
                    bc = sb.tile([P, 2, HC], f32, tag="bc")
                    nc.vector.tensor_copy(bc[:].rearrange("p a c -> p (a c)"),
                                          bc_ps[:])
                    h_t = unorm
                    nc.vector.tensor_tensor(
                        out=h_t[:], in0=unorm[:],
                        in1=bc[:, 0:1, :].to_broadcast([P, nblk, HC]),
                        op=mybir.AluOpType.mult)
                    nc.vector.tensor_tensor(
                        out=h_t[:], in0=h_t[:],
                        in1=bc[:, 1:2, :].to_broadcast([P, nblk, HC]),
                        op=mybir.AluOpType.add)
                    nc.vector.tensor_scalar(out=h_t[:], in0=h_t[:], scalar1=0.0,
                                            scalar2=None, op0=mybir.AluOpType.max)
                    for b in range(nblk):
                        for fi in range(HC // P):
                            t_ps = ps2.tile([P, P], bf16, tag="tps")
                            nc.tensor.transpose(t_ps[:],
                                                h_t[:, b, fi * P:(fi + 1) * P],
                                                ident[:])
                            if (b + fi) % 2 == 0:
                                nc.vector.tensor_copy(
                                    hT[:, fi, b * P:(b + 1) * P], t_ps[:])
                            else:
                                nc.scalar.copy(hT[:, fi, b * P:(b + 1) * P], t_ps[:])
                else:
                    # ---- pool + final ----
                    b3_sb = sb.tile([1, HID], f32, tag="b3sb")
                    nc.sync.dma_start(b3_sb[:], b3row[:])
                    p01_sb = con.tile([P, nblk, cfg.G], bf16, tag="p01sb")
                    nc.sync.dma_start(p01_sb[:], p01[:].rearrange("b p g -> p b g"))
                    pl_ps = ps.tile([cfg.G, HID], f32, tag="gemm")
                    for b in range(nblk):
                        nc.tensor.matmul(pl_ps[:], p01_sb[:, b, :],
                                         unorm[:, b, 0:HID], start=(b == 0),
                                         stop=(b == nblk - 1))
                    pl_sb = sb.tile([cfg.G, HID], f32, tag="plsb")
                    nc.vector.tensor_copy(pl_sb[:], pl_ps[:])
                    nc.sync.dma_start(pool_loc[:], pl_sb[:])
                    nc.gpsimd.collective_compute(
                        "AllReduce", mybir.AluOpType.add, replica_groups=groups,
                        ins=[pool_loc[:].opt()], outs=[pool_full[:].opt()])
                    pl2 = sb.tile([cfg.G, HID], f32, tag="pl2")
                    nc.sync.dma_start(pl2[:], pool_full[:])
                    ic_sb = sb.tile([cfg.G, 1], f32, tag="icsb")
                    nc.sync.dma_start(ic_sb[:], inv_cnt[:])
                    nc.vector.tensor_scalar(out=pl2[:], in0=pl2[:],
                                            scalar1=ic_sb[:], scalar2=None,
                                            op0=mybir.AluOpType.mult)
                    onesG = sb.tile([1, cfg.G], bf16, tag="onesG")
                    nc.gpsimd.memset(onesG[:], 1.0)
                    b3b = sb.tile([1, HID], bf16, tag="b3b")
                    nc.vector.tensor_copy(b3b[:], b3_sb[:])
                    b3bc_ps = ps2.tile([cfg.G, HID], f32, tag="edps")
                    nc.tensor.matmul(b3bc_ps[:], onesG[:], b3b[:],
                                     start=True, stop=True)
                    nc.vector.tensor_tensor(out=pl2[:], in0=pl2[:], in1=b3bc_ps[:],
                                            op=mybir.AluOpType.add)
                    plb = sb.tile([cfg.G, HID], bf16, tag="plb")
                    nc.vector.tensor_copy(plb[:], pl2[:])
                    pT_ps = ps2.tile([P, cfg.G], bf16, tag="tps")
                    nc.tensor.transpose(pT_ps[0:HID, 0:cfg.G], plb[:], ident[0:cfg.G, 0:cfg.G])
                    pT = sb.tile([P, cfg.G], bf16, tag="pT")
                    nc.vector.tensor_copy(pT[0:HID, :], pT_ps[0:HID, :])
                    wl_sb = sb.tile([P, cfg.OUT], bf16, tag="wlsb")
                    nc.sync.dma_start(wl_sb[0:HID, :], wlin[:])
                    fin_ps = ps.tile([cfg.G, cfg.OUT], f32, tag="gemm")
                    nc.tensor.matmul(fin_ps[:], pT[0:HID, 0:cfg.G],
                                     wl_sb[0:HID, :], start=True, stop=True)
                    bl_sb = sb.tile([1, cfg.OUT], f32, tag="blsb")
                    nc.sync.dma_start(bl_sb[:], blinrow[:])
                    blb = sb.tile([1, cfg.OUT], bf16, tag="blb")
                    nc.vector.tensor_copy(blb[:], bl_sb[:])
                    blbc_ps = ps2.tile([cfg.G, cfg.OUT], f32, tag="edps")
                    nc.tensor.matmul(blbc_ps[:], onesG[:], blb[:],
                                     start=True, stop=True)
                    fin_sb = sb.tile([cfg.G, cfg.OUT], f32, tag="finsb")
                    nc.vector.tensor_copy(fin_sb[:], blbc_ps[:])
                    nc.vector.tensor_tensor(out=fin_sb[:], in0=fin_ps[:],
                                            in1=fin_sb[:], op=mybir.AluOpType.add)
                    nc.sync.dma_start(out_t[:], fin_sb[:])
    nc.finalize()
    return nc


# ======================= harness entry =======================
_CACHE = {}


def _install_ntff_hook():
    try:
        import types
        if "antenv.axon_hooks" in sys.modules:
            return
        import antenv
        mod = types.ModuleType("antenv.axon_hooks")
        _state = {"hook": None}
        mod.set_axon_ntff_profile_hook = lambda h: _state.__setitem__("hook", h)
        mod.get_axon_ntff_profile_hook = lambda: _state["hook"]
        sys.modules["antenv.axon_hooks"] = mod
        antenv.axon_hooks = mod
        from trn_agent_boot.trn_boot import _ntff_profile_via_ctypes
        mod.set_axon_ntff_profile_hook(
            _ntff_profile_via_ctypes("/opt/axon/libaxon_pjrt.so"))
    except Exception:
        pass


def kernel(**inputs):
    import os
    from concourse import bass_utils
    x = np.asarray(inputs["x"], np.float32)
    ei = np.asarray(inputs["edge_index"], np.int64)
    batch = np.asarray(inputs["batch"], np.int64)
    cfg = Cfg(N=x.shape[0], E=ei.shape[1], G=64, IN=x.shape[1], HID=128,
              HEADS=2, OUT=64, ncores=8)
    key = (ei.tobytes(), batch.tobytes())
    kh = hash(key)
    if kh in _CACHE:
        per_core, meta, nc = _CACHE[kh]
    else:
        per_core, meta = host_prep(cfg, ei, batch)
        nc = build_kernel(cfg, meta)
        _CACHE[kh] = (per_core, meta, nc)
    wts = build_weights(
        inputs["W1"], inputs["as1"], inputs["ad1"], inputs["W2"],
        inputs["as2"], inputs["ad2"], inputs["W3"], inputs["as3"],
        inputs["ad3"], inputs["Wlin"], inputs["b3"], inputs["blin"],
        inputs["g1"], inputs["be1"], inputs["g2"], inputs["be2"])
    in_maps = []
    for c in range(cfg.ncores):
        d = dict(per_core[c])
        d.update(wts)
        xs = x[c * cfg.npc:(c + 1) * cfg.npc]
        if cfg.npc_pad > cfg.npc:
            xs = np.pad(xs, ((0, cfg.npc_pad - cfg.npc), (0, 0)))
        d["xT"] = np.ascontiguousarray(xs.T).astype(ml_dtypes.bfloat16)
        in_maps.append(d)
    trace = os.environ.get("GAT_TRACE", "0") == "1"
    if trace:
        _install_ntff_hook()
    res = bass_utils.run_bass_kernel_spmd(
        nc, in_maps, core_ids=list(range(cfg.ncores)), trace=trace)
    if trace and res.exec_time_ns is not None:
        print(f"HW exec time: {res.exec_time_ns} ns")
        kernel.last_exec_time_ns = res.exec_time_ns
        kernel.last_scope_times = res.per_core_scope_times
        kernel.last_trace = res.instructions_and_trace
    return np.asarray(res.results[0]["out"], np.float32)



# revision 4
# speedup vs baseline: 1.0713x; 1.0713x over previous
"""ComplexGAT Trainium2 kernel: builder + host preprocessing.

Sharding: dst-node blocks across 8 cores (SPMD, one program). Per layer:
  GEMM (W augmented with [a_src|a_dst] cols -> es/ed free) -> node-major table
  rows [xl | es | pad] -> AllGather -> per dst-block: dma_gather the block's
  edge-chunk window (768B rows, es rides), ed via S01^T matmuls (fp8 host
  patterns), w = exp(lrelu(es+ed)) on ACT, rhs = [G*w | w], scatter via
  fp8 S01 matmuls accumulating [u | den] in PSUM; self-loops are folded in
  locally (w_self * xl added at normalize time -> no gather descriptors).
  Nodes are re-packed into blocks on the host to balance per-block per-half
  edge counts (cuts chunk padding).
  BN: mask-matmul stats + AllReduce + scale/shift broadcast via ones-matmul,
  relu; PE transpose-back for next GEMM lhsT. Final: pool matmul + AllReduce
  + linear.
"""
import sys
import numpy as np
import ml_dtypes

sys.path.insert(0, "/opt/trn_rl_repo")
import concourse.bass as bass
import concourse.bacc as bacc
import concourse.mybir as mybir
import concourse.tile as tile
from concourse.masks import make_identity

P = 128
bf16 = mybir.dt.bfloat16
fp8 = mybir.dt.float8e4
f32 = mybir.dt.float32
NEG_SLOPE = 0.2
BN_EPS = 1e-5


class Cfg:
    def __init__(self, N=50000, E=800000, G=64, IN=512, HID=128, HEADS=2,
                 OUT=64, ncores=8):
        self.N, self.E, self.G, self.IN, self.HID, self.HEADS, self.OUT = \
            N, E, G, IN, HID, HEADS, OUT
        self.ncores = ncores
        self.npc = N // ncores
        assert self.npc * ncores == N
        self.nblk = (self.npc + P - 1) // P
        self.npc_pad = self.nblk * P
        self.Npad = self.npc_pad * ncores
        self.HC = HID * HEADS
        self.TW = 384 if HEADS == 2 else 256   # table row elems (bytes%256==0)


def host_prep(cfg, edge_index, batch):
    N, ncores, npc, nblk = cfg.N, cfg.ncores, cfg.npc, cfg.nblk
    src = np.asarray(edge_index[0], np.int64)
    dst = np.asarray(edge_index[1], np.int64)
    half = cfg.Npad // 2
    core_src = src // npc
    core_dst = dst // npc
    src_half = ((core_src * cfg.npc_pad) >= half).astype(np.int64)

    # ---- per-core node -> (block, slot) packing balancing per-half loads
    slot_of = np.full(N, -1, np.int64)
    orders = []
    for c in range(ncores):
        em = core_dst == c
        d_loc = dst[em] - c * npc
        sh_e = src_half[em]
        d0 = np.bincount(d_loc[sh_e == 0], minlength=npc)
        d1 = np.bincount(d_loc[sh_e == 1], minlength=npc)
        # per-half edge caps: 8 chunks/block, first m blocks absorb the excess
        caps = []
        for dd in (d0, d1):
            T = int(dd.sum())
            m = max(0, -(-(T - nblk * 8 * P) // P))
            cap = np.full(nblk, 8 * P, np.int64)
            cap[:min(m, nblk)] += P
            caps.append(cap)
        cap0, cap1 = caps
        idx_sorted = np.argsort(-(d0 + d1), kind="stable")
        capn = np.full(nblk, P, np.int64)
        l0 = np.zeros(nblk, np.int64)
        l1 = np.zeros(nblk, np.int64)
        blk_of = np.zeros(npc, np.int64)
        for n_loc in idx_sorted:
            over = (np.maximum(l0 + d0[n_loc] - cap0, 0)
                    + np.maximum(l1 + d1[n_loc] - cap1, 0))
            r = np.minimum(cap0 - l0 - d0[n_loc], cap1 - l1 - d1[n_loc])
            score = over * (1 << 20) - r
            score[capn == 0] = 1 << 60
            b = int(np.argmin(score))
            blk_of[n_loc] = b
            l0[b] += d0[n_loc]
            l1[b] += d1[n_loc]
            capn[b] -= 1
        # refinement: swap nodes out of over-cap blocks into slack blocks
        for _ in range(800):
            ov0 = l0 - cap0
            ov1 = l1 - cap1
            ovt = np.maximum(ov0, 0) + np.maximum(ov1, 0)
            if ovt.max() <= 0:
                break
            b_bad = int(np.argmax(ovt))
            n0, n1 = int(ov0[b_bad] > 0), int(ov1[b_bad] > 0)
            in_bad = np.where(blk_of == b_bad)[0]
            don = in_bad[np.argsort(-(d0[in_bad] * n0 + d1[in_bad] * n1))][:6]
            best = None
            for i in don:
                b2v = blk_of
                nl0 = l0[b2v] - d0 + d0[i]
                nl1 = l1[b2v] - d1 + d1[i]
                feas = (nl0 <= cap0[b2v]) & (nl1 <= cap1[b2v]) & (b2v != b_bad)
                t0 = np.maximum(l0[b_bad] - d0[i] + d0 - cap0[b_bad], 0)
                t1 = np.maximum(l1[b_bad] - d1[i] + d1 - cap1[b_bad], 0)
                tot = t0 + t1
                tot[~feas] = 1 << 30
                j = int(np.argmin(tot))
                if tot[j] < (best[0] if best else ovt[b_bad]):
                    best = (int(tot[j]), int(i), j)
                    if tot[j] == 0:
                        break
            if best is None:
                break
            _, i, j = best
            bi, bj = int(blk_of[i]), int(blk_of[j])
            blk_of[i], blk_of[j] = bj, bi
            l0[bi] += d0[j] - d0[i]
            l1[bi] += d1[j] - d1[i]
            l0[bj] += d0[i] - d0[j]
            l1[bj] += d1[i] - d1[j]
        order = np.full(cfg.npc_pad, -1, np.int64)
        fill = np.zeros(nblk, np.int64)
        for n_loc in range(npc):
            b = int(blk_of[n_loc])
            s = b * P + int(fill[b])
            fill[b] += 1
            slot_of[c * npc + n_loc] = s
            order[s] = c * npc + n_loc
        orders.append(order)
    rowid = (np.arange(N) // npc) * cfg.npc_pad + slot_of

    # ---- per (core, li, block) edge groups; per-block window widths
    Kmat = np.zeros((2, nblk), np.int64)
    groups_e = {}
    for c in range(ncores):
        em = core_dst == c
        for li in range(2):
            m2 = em & (src_half == li)
            s_r = rowid[src[m2]] - li * half
            d_s = slot_of[dst[m2]]
            blk = d_s // P
            for b in range(nblk):
                bm = blk == b
                groups_e[(c, li, b)] = (s_r[bm], d_s[bm])
                Kmat[li, b] = max(Kmat[li, b], (bm.sum() + P - 1) // P)
    win = np.zeros((2, nblk, 2), np.int64)
    for li in range(2):
        pos = 0
        for b in range(nblk):
            win[li, b] = (pos, pos + Kmat[li, b])
            pos += Kmat[li, b]
    Cn = [int(Kmat[0].sum()), int(Kmat[1].sum())]
    wmax = int(Kmat.max())
    nch = [Cn[0], Cn[1]]

    per_core = []
    for c in range(ncores):
        d = {}
        for li in range(2):
            Cl = Cn[li]
            idxs = np.zeros(Cl * P, np.int64)
            dloc = np.full(Cl * P, -1, np.int64)
            for b in range(nblk):
                s_b, d_b = groups_e[(c, li, b)]
                e0 = int(win[li, b, 0]) * P
                idxs[e0:e0 + s_b.size] = s_b
                dloc[e0:e0 + s_b.size] = d_b
            assert idxs.min() >= 0 and (Cl == 0 or idxs.max() < half)
            iw = np.zeros((16, Cl * P // 16), np.int16)
            ar = np.arange(Cl * P)
            iw[ar % 16, ar // 16] = idxs
            d[f"idx{li}"] = np.tile(iw, (8, 1))
            S = np.zeros((max(Cl, 1), P, P), np.float32)
            ST = np.zeros((max(Cl, 1), P, P), np.float32)
            rows = np.arange(P)
            for b in range(nblk):
                for j in range(int(Kmat[li, b])):
                    ch = int(win[li, b, 0]) + j
                    dl = dloc[ch * P:(ch + 1) * P]
                    ok = (dl >= b * P) & (dl < (b + 1) * P)
                    S[ch, rows[ok], dl[ok] - b * P] = 1.0
                    ST[ch] = S[ch].T
            d[f"s01_{li}"] = np.ascontiguousarray(
                S.transpose(1, 0, 2)).astype(ml_dtypes.float8_e4m3)
            d[f"s01t_{li}"] = np.ascontiguousarray(
                ST.transpose(1, 0, 2)).astype(ml_dtypes.float8_e4m3)
        per_core.append(d)

    Gn = cfg.G
    batch = np.asarray(batch, np.int64)
    cnt = np.bincount(batch, minlength=Gn).astype(np.float64)
    inv_cnt = (1.0 / np.maximum(cnt, 1.0)).astype(np.float32)[:, None]
    for c in range(ncores):
        order = orders[c]
        p01 = np.zeros((nblk, P, Gn), np.float32)
        mask = np.zeros((P, nblk), np.float32)
        for s in range(cfg.npc_pad):
            node = order[s]
            if node >= 0:
                p01[s // P, s % P, batch[node]] = 1.0
                mask[s % P, s // P] = 1.0
        per_core[c]["p01"] = p01.astype(ml_dtypes.bfloat16)
        per_core[c]["mask"] = mask.astype(ml_dtypes.bfloat16)
        per_core[c]["inv_cnt"] = inv_cnt
        per_core[c]["order"] = order
    meta = {"Cn": Cn, "win": win, "wmax": wmax, "nch": nch}
    return per_core, meta


def build_weights(W1, as1, ad1, W2, as2, ad2, W3, as3, ad3, Wlin, b3, blin,
                  g1, be1, g2, be2):
    def aug(W, a_s, a_d):
        H, C = np.asarray(a_s).shape
        A = np.zeros((W.shape[1], 2 * H), np.float32)
        for h in range(H):
            A[h * C:(h + 1) * C, h] = a_s[h]
            A[h * C:(h + 1) * C, H + h] = a_d[h]
        return np.concatenate([np.asarray(W, np.float32),
                               np.asarray(W, np.float32) @ A], 1)
    return {
        "w1a": aug(W1, as1, ad1).astype(ml_dtypes.bfloat16),
        "w2a": aug(W2, as2, ad2).astype(ml_dtypes.bfloat16),
        "w3a": aug(W3, as3, ad3).astype(ml_dtypes.bfloat16),
        "wlin": np.asarray(Wlin).astype(ml_dtypes.bfloat16),
        "bn1": np.stack([g1, be1]).astype(np.float32),
        "bn2": np.stack([g2, be2]).astype(np.float32),
        "b3row": np.asarray(b3)[None, :].astype(np.float32),
        "blinrow": np.asarray(blin)[None, :].astype(np.float32),
    }


def build_kernel(cfg, meta):
    nc = bacc.Bacc("TRN2", target_bir_lowering=False, debug=False,
                   num_devices=cfg.ncores, enable_asserts=False)
    HC, HID, HEADS = cfg.HC, cfg.HID, cfg.HEADS
    nblk = cfg.nblk
    Cn, win, wmax, nch = meta["Cn"], meta["win"], meta["wmax"], meta["nch"]
    NPP = cfg.npc_pad
    TW = cfg.TW
    half_rows = cfg.Npad // 2
    KMAX = max(cfg.IN, HC) // P

    xT = nc.dram_tensor("xT", [cfg.IN, NPP], bf16, kind="ExternalInput")
    w1a = nc.dram_tensor("w1a", [cfg.IN, HC + 2 * HEADS], bf16, kind="ExternalInput")
    w2a = nc.dram_tensor("w2a", [HC, HC + 2 * HEADS], bf16, kind="ExternalInput")
    w3a = nc.dram_tensor("w3a", [HC, HID + 2], bf16, kind="ExternalInput")
    wlin = nc.dram_tensor("wlin", [HID, cfg.OUT], bf16, kind="ExternalInput")
    bn1 = nc.dram_tensor("bn1", [2, HC], f32, kind="ExternalInput")
    bn2 = nc.dram_tensor("bn2", [2, HC], f32, kind="ExternalInput")
    b3row = nc.dram_tensor("b3row", [1, HID], f32, kind="ExternalInput")
    blinrow = nc.dram_tensor("blinrow", [1, cfg.OUT], f32, kind="ExternalInput")
    idx_t = [nc.dram_tensor(f"idx{li}", [P, Cn[li] * P // 16], mybir.dt.int16,
                            kind="ExternalInput") for li in range(2)]
    s01_t = [nc.dram_tensor(f"s01_{li}", [P, max(nch[li], 1), P], fp8,
                            kind="ExternalInput") for li in range(2)]
    s01t_t = [nc.dram_tensor(f"s01t_{li}", [P, max(nch[li], 1), P], fp8,
                             kind="ExternalInput") for li in range(2)]
    p01 = nc.dram_tensor("p01", [nblk, P, cfg.G], bf16, kind="ExternalInput")
    maskv = nc.dram_tensor("mask", [P, nblk], bf16, kind="ExternalInput")
    inv_cnt = nc.dram_tensor("inv_cnt", [cfg.G, 1], f32, kind="ExternalInput")
    out_t = nc.dram_tensor("out", [cfg.G, cfg.OUT], f32, kind="ExternalOutput")

    tab_loc = nc.dram_tensor("tab_loc", [NPP, TW], bf16)
    shr = "Shared" if cfg.ncores > 4 else "Local"
    tab_full = nc.dram_tensor("tab_full", [cfg.Npad, TW], bf16, addr_space=shr)
    stats_loc = nc.dram_tensor("stats_loc", [1, 2 * HC], f32)
    stats_full = nc.dram_tensor("stats_full", [1, 2 * HC], f32, addr_space=shr)
    pool_loc = nc.dram_tensor("pool_loc", [cfg.G, HID], f32)
    pool_full = nc.dram_tensor("pool_full", [cfg.G, HID], f32, addr_space=shr)
    groups = [list(range(cfg.ncores))]

    with tile.TileContext(nc) as tc:
        with tc.tile_pool(name="con", bufs=1) as con, \
             tc.tile_pool(name="sb", bufs=2) as sb, \
             tc.tile_pool(name="blk", bufs=2) as bp, \
             tc.tile_pool(name="ps", bufs=2, space="PSUM") as ps, \
             tc.tile_pool(name="ps2", bufs=2, space="PSUM") as ps2:

            hT = con.tile([P, HC // P, NPP], bf16, tag="hT")
            idx_sb = []
            for li in range(2):
                t_idx = con.tile([P, Cn[li] * P // 16], mybir.dt.int16,
                                 tag=f"idx{li}")
                idx_sb.append(t_idx)
            for li in range(2):
                nc.sync.dma_start(idx_sb[li][:], idx_t[li][:])
            mask_sb = con.tile([P, nblk], bf16, tag="mask")
            nc.sync.dma_start(mask_sb[:], maskv[:])
            ones_sb = con.tile([1, P], bf16, tag="ones")
            nc.gpsimd.memset(ones_sb[:], 1.0)
            ident = con.tile([P, P], bf16, tag="ident")
            make_identity(nc, ident[:])
            unorm = con.tile([P, nblk, HC], bf16, tag="unorm")
            edloc = con.tile([P, nblk, 2 * HEADS], bf16, tag="edloc")
            wself = con.tile([P, nblk, HEADS], f32, tag="wself")

            for layer in range(3):
                K_in = cfg.IN if layer == 0 else HC
                M = HC if layer < 2 else HID
                H = HEADS if layer < 2 else 1
                C_h = M // H
                Wt = (w1a, w2a, w3a)[layer]
                wcols = M + 2 * H
                # ---- GEMM + table stage ----
                stage_b = con.tile([P, TW], bf16, tag="stage_b")
                nc.gpsimd.memset(stage_b[:], 0.0)
                w_sb = sb.tile([P, KMAX, HC + 2 * HEADS], bf16, tag="wsb")
                nc.sync.dma_start(w_sb[:, 0:K_in // P, 0:wcols],
                                  Wt[:].rearrange("(k p) m -> p k m", p=P))
                for b in range(nblk):
                    if layer == 0:
                        lhs_b = sb.tile([P, K_in // P, P], bf16, tag="lhsb")
                        nc.sync.dma_start(
                            lhs_b[:],
                            xT[:].rearrange("(k p) n -> p k n", p=P)
                            [:, :, b * P:(b + 1) * P])
                    g_ps = ps.tile([P, HC + 2 * HEADS], f32, tag="gemm")
                    for k in range(K_in // P):
                        lhsT_ap = (lhs_b[:, k, :] if layer == 0
                                   else hT[:, k, b * P:(b + 1) * P])
                        nc.tensor.matmul(g_ps[:, 0:wcols], lhsT_ap,
                                         w_sb[:, k, 0:wcols],
                                         start=(k == 0), stop=(k == K_in // P - 1))
                    nc.vector.tensor_copy(stage_b[:, 0:M + H], g_ps[:, 0:M + H])
                    nc.scalar.copy(edloc[:, b, 0:2 * H], g_ps[:, M:M + 2 * H])
                    nc.sync.dma_start(tab_loc[b * P:(b + 1) * P, :], stage_b[:])
                nc.gpsimd.collective_compute(
                    "AllGather", mybir.AluOpType.bypass, replica_groups=groups,
                    ins=[tab_loc[:].opt()], outs=[tab_full[:].opt()])

                # self-loop weights: w_self = exp(lrelu(es + ed)) per node
                zs = sb.tile([P, nblk, HEADS], f32, tag="zself")
                nc.vector.tensor_tensor(out=zs[:, :, 0:H],
                                        in0=edloc[:, :, 0:H],
                                        in1=edloc[:, :, H:2 * H],
                                        op=mybir.AluOpType.add)
                nc.scalar.activation(wself[:, :, 0:H], zs[:, :, 0:H],
                                     mybir.ActivationFunctionType.Lrelu,
                                     alpha=NEG_SLOPE)
                nc.scalar.activation(wself[:, :, 0:H], wself[:, :, 0:H],
                                     mybir.ActivationFunctionType.Exp)

                # ---- per dst-block ----
                sptr = [0, 0]
                for b in range(nblk):
                    wins = [(int(win[li, b, 0]), int(win[li, b, 1]))
                            for li in range(2)]
                    Wb = [c1 - c0 for (c0, c1) in wins]
                    xlb = bp.tile([P, HC], bf16, tag="xlb")
                    nc.sync.dma_start(xlb[:, 0:M],
                                      tab_loc[b * P:(b + 1) * P, 0:M])
                    o_ps = ps.tile([P, HC + HEADS], f32, tag="ops")
                    first = True
                    for li in range(2):
                        c0, c1 = wins[li]
                        if Wb[li] == 0:
                            continue
                        G_t = bp.tile([P, wmax, TW], bf16, tag=f"G{li}")
                        nc.gpsimd.dma_gather(
                            out_ap=G_t[:, 0:Wb[li], :],
                            in_ap=tab_full[li * half_rows:(li + 1) * half_rows, :],
                            idxs_ap=idx_sb[li][:, c0 * 8:c1 * 8],
                            num_idxs=Wb[li] * P, num_idxs_reg=Wb[li] * P,
                            elem_size=TW, single_packet=False)
                        # ed matmuls (one per chunk) -> psum
                        st_sb = bp.tile([P, wmax, P], fp8, tag=f"st{li}")
                        nc.sync.dma_start(
                            st_sb[:, 0:Wb[li], :],
                            s01t_t[li][:, sptr[li]:sptr[li] + Wb[li], :])
                        ed_ps = ps2.tile([P, wmax, H], f32, tag="edps")
                        for j in range(Wb[li]):
                            nc.tensor.matmul(ed_ps[:, j, :], st_sb[:, j, :],
                                             edloc[:, b, H:2 * H],
                                             start=True, stop=True)
                        # z = es + ed ; w = exp(lrelu(z)) on ACT
                        z_t = bp.tile([P, wmax, 2 * H], f32, tag=f"z{li}")
                        rhs_t = bp.tile([P, wmax, HC + HEADS], bf16, tag=f"rhs{li}")
                        nc.vector.tensor_tensor(
                            out=z_t[:, 0:Wb[li], 0:H],
                            in0=G_t[:, 0:Wb[li], M:M + H],
                            in1=ed_ps[:, 0:Wb[li], :], op=mybir.AluOpType.add)
                        nc.scalar.activation(z_t[:, 0:Wb[li], H:2 * H],
                                             z_t[:, 0:Wb[li], 0:H],
                                             mybir.ActivationFunctionType.Lrelu,
                                             alpha=NEG_SLOPE)
                        nc.scalar.activation(rhs_t[:, 0:Wb[li], M:M + H],
                                             z_t[:, 0:Wb[li], H:2 * H],
                                             mybir.ActivationFunctionType.Exp)
                        for h in range(H):
                            nc.vector.tensor_tensor(
                                out=rhs_t[:, 0:Wb[li], h * C_h:(h + 1) * C_h],
                                in0=G_t[:, 0:Wb[li], h * C_h:(h + 1) * C_h],
                                in1=rhs_t[:, 0:Wb[li], M + h:M + h + 1]
                                .to_broadcast([P, Wb[li], C_h]),
                                op=mybir.AluOpType.mult)
                        # scatter matmuls
                        s_sb = bp.tile([P, wmax, P], fp8, tag=f"s{li}")
                        nc.sync.dma_start(
                            s_sb[:, 0:Wb[li], :],
                            s01_t[li][:, sptr[li]:sptr[li] + Wb[li], :])
                        sptr[li] += Wb[li]
                        for j in range(Wb[li]):
                            last = (li == (1 if Wb[1] else 0)) and (j == Wb[li] - 1)
                            nc.tensor.matmul(o_ps[:, 0:M + H], s_sb[:, j, :],
                                             rhs_t[:, j, 0:M + H],
                                             start=first, stop=last)
                            first = False
                    # normalize, folding in the self-loop term
                    rec = sb.tile([P, HEADS], f32, tag="rec")
                    nc.vector.tensor_tensor(out=rec[:, 0:H],
                                            in0=o_ps[:, M:M + H],
                                            in1=wself[:, b, 0:H],
                                            op=mybir.AluOpType.add)
                    nc.vector.reciprocal(rec[:, 0:H], rec[:, 0:H])
                    ut = sb.tile([P, HC], f32, tag="ut")
                    for h in range(H):
                        nc.vector.tensor_tensor(
                            out=ut[:, h * C_h:(h + 1) * C_h],
                            in0=xlb[:, h * C_h:(h + 1) * C_h],
                            in1=wself[:, b, h:h + 1].to_broadcast([P, C_h]),
                            op=mybir.AluOpType.mult)
                    nc.vector.tensor_tensor(out=ut[:, 0:M], in0=ut[:, 0:M],
                                            in1=o_ps[:, 0:M],
                                            op=mybir.AluOpType.add)
                    for h in range(H):
                        nc.vector.tensor_tensor(
                            out=unorm[:, b, h * C_h:(h + 1) * C_h],
                            in0=ut[:, h * C_h:(h + 1) * C_h],
                            in1=rec[:, h:h + 1].to_broadcast([P, C_h]),
                            op=mybir.AluOpType.mult)

                if layer < 2:
                    # ---- BN + relu + transpose-back ----
                    st_ps = ps.tile([1, 2, HC], f32, tag="gemm")
                    for b in range(nblk):
                        nc.tensor.matmul(st_ps[:, 0, :], mask_sb[:, b:b + 1],
                                         unorm[:, b, :], start=(b == 0),
                                         stop=(b == nblk - 1))
                    for b in range(nblk):
                        sq_b = sb.tile([P, HC], bf16, tag="sq_b")
                        nc.scalar.activation(sq_b[:], unorm[:, b, :],
                                             mybir.ActivationFunctionType.Square)
                        nc.tensor.matmul(st_ps[:, 1, :], mask_sb[:, b:b + 1],
                                         sq_b[:], start=(b == 0),
                                         stop=(b == nblk - 1))
                    st_sb2 = sb.tile([1, 2, HC], f32, tag="stsb")
                    nc.vector.tensor_copy(st_sb2[:], st_ps[:])
                    nc.sync.dma_start(stats_loc[:],
                                      st_sb2[:].rearrange("a b c -> a (b c)"))
                    nc.gpsimd.collective_compute(
                        "AllReduce", mybir.AluOpType.add, replica_groups=groups,
                        ins=[stats_loc[:].opt()], outs=[stats_full[:].opt()])
                    st2 = sb.tile([1, 2, HC], f32, tag="st2")
                    nc.sync.dma_start(st2[:].rearrange("a b c -> a (b c)"),
                                      stats_full[:])
                    bnp = sb.tile([1, 2, HC], f32, tag="bnp")
                    nc.sync.dma_start(bnp[:].rearrange("a b c -> a (b c)"),
                                      (bn1 if layer == 0 else bn2)[:].rearrange(
                                          "b c -> (b c)").rearrange(
                                          "(o k) -> o k", o=1))
                    tmp = sb.tile([1, 2, HC], f32, tag="tmp")
                    sc = sb.tile([1, 2, HC], f32, tag="sc")
                    # tmp = [mu, Ex2]
                    nc.vector.tensor_scalar(out=tmp[:], in0=st2[:],
                                            scalar1=1.0 / cfg.N, scalar2=None,
                                            op0=mybir.AluOpType.mult)
                    # sc[0] = var = Ex2 - mu^2
                    nc.vector.tensor_tensor(out=sc[:, 0, :], in0=tmp[:, 0, :],
                                            in1=tmp[:, 0, :], op=mybir.AluOpType.mult)
                    nc.vector.tensor_tensor(out=sc[:, 0, :], in0=tmp[:, 1, :],
                                            in1=sc[:, 0, :],
                                            op=mybir.AluOpType.subtract)
                    nc.vector.tensor_scalar(out=sc[:, 0, :], in0=sc[:, 0, :],
                                            scalar1=BN_EPS, scalar2=None,
                                            op0=mybir.AluOpType.add)
                    nc.scalar.activation(sc[:, 1, :], sc[:, 0, :],
                                         mybir.ActivationFunctionType.Sqrt)
                    nc.vector.reciprocal(sc[:, 1, :], sc[:, 1, :])
                    # sc[0] = scale = g * rsqrt ; sc[1] = shift = beta - mu*scale
                    nc.vector.tensor_tensor(out=sc[:, 0, :], in0=bnp[:, 0, :],
                                            in1=sc[:, 1, :], op=mybir.AluOpType.mult)
                    nc.vector.tensor_tensor(out=tmp[:, 0, :], in0=tmp[:, 0, :],
                                            in1=sc[:, 0, :], op=mybir.AluOpType.mult)
                    nc.vector.tensor_tensor(out=sc[:, 1, :], in0=bnp[:, 1, :],
                                            in1=tmp[:, 0, :],
                                            op=mybir.AluOpType.subtract)
                    scb = sb.tile([1, 2, HC], bf16, tag="scb")
                    nc.vector.tensor_copy(scb[:], sc[:])
                    bc_ps = ps.tile([P, 2 * HC], f32, tag="gemm")
                    nc.tensor.matmul(bc_ps[:], ones_sb[:],
                                     scb[:].rearrange("a b c -> a (b c)"),
                                     start=True, stop=True)
                    bc = sb.tile([P, 2, HC], f32, tag="bc")
                    nc.vector.tensor_copy(bc[:].rearrange("p a c -> p (a c)"),
                                          bc_ps[:])
                    h_t = unorm
                    nc.vector.tensor_tensor(
                        out=h_t[:], in0=unorm[:],
                        in1=bc[:, 0:1, :].to_broadcast([P, nblk, HC]),
                        op=mybir.AluOpType.mult)
                    nc.vector.tensor_tensor(
                        out=h_t[:], in0=h_t[:],
                        in1=bc[:, 1:2, :].to_broadcast([P, nblk, HC]),
                        op=mybir.AluOpType.add)
                    nc.vector.tensor_scalar(out=h_t[:], in0=h_t[:], scalar1=0.0,
                                            scalar2=None, op0=mybir.AluOpType.max)
                    for b in range(nblk):
                        for fi in range(HC // P):
                            t_ps = ps2.tile([P, P], bf16, tag="tps")
                            nc.tensor.transpose(t_ps[:],
                                                h_t[:, b, fi * P:(fi + 1) * P],
                                                ident[:])
                            if (b + fi) % 2 == 0:
                                nc.vector.tensor_copy(
                                    hT[:, fi, b * P:(b + 1) * P], t_ps[:])
                            else:
                                nc.scalar.copy(hT[:, fi, b * P:(b + 1) * P], t_ps[:])
                else:
                    # ---- pool + final ----
                    b3_sb = sb.tile([1, HID], f32, tag="b3sb")
                    nc.sync.dma_start(b3_sb[:], b3row[:])
                    p01_sb = con.tile([P, nblk, cfg.G], bf16, tag="p01sb")
                    nc.sync.dma_start(p01_sb[:], p01[:].rearrange("b p g -> p b g"))
                    pl_ps = ps.tile([cfg.G, HID], f32, tag="gemm")
                    for b in range(nblk):
                        nc.tensor.matmul(pl_ps[:], p01_sb[:, b, :],
                                         unorm[:, b, 0:HID], start=(b == 0),
                                         stop=(b == nblk - 1))
                    pl_sb = sb.tile([cfg.G, HID], f32, tag="plsb")
                    nc.vector.tensor_copy(pl_sb[:], pl_ps[:])
                    nc.sync.dma_start(pool_loc[:], pl_sb[:])
                    nc.gpsimd.collective_compute(
                        "AllReduce", mybir.AluOpType.add, replica_groups=groups,
                        ins=[pool_loc[:].opt()], outs=[pool_full[:].opt()])
                    pl2 = sb.tile([cfg.G, HID], f32, tag="pl2")
                    nc.sync.dma_start(pl2[:], pool_full[:])
                    ic_sb = sb.tile([cfg.G, 1], f32, tag="icsb")
                    nc.sync.dma_start(ic_sb[:], inv_cnt[:])
                    nc.vector.tensor_scalar(out=pl2[:], in0=pl2[:],
                                            scalar1=ic_sb[:], scalar2=None,
                                            op0=mybir.AluOpType.mult)
                    onesG = sb.tile([1, cfg.G], bf16, tag="onesG")
                    nc.gpsimd.memset(onesG[:], 1.0)
                    b3b = sb.tile([1, HID], bf16, tag="b3b")
                    nc.vector.tensor_copy(b3b[:], b3_sb[:])
                    b3bc_ps = ps2.tile([cfg.G, HID], f32, tag="edps")
                    nc.tensor.matmul(b3bc_ps[:], onesG[:], b3b[:],
                                     start=True, stop=True)
                    nc.vector.tensor_tensor(out=pl2[:], in0=pl2[:], in1=b3bc_ps[:],
                                            op=mybir.AluOpType.add)
                    plb = sb.tile([cfg.G, HID], bf16, tag="plb")
                    nc.vector.tensor_copy(plb[:], pl2[:])
                    pT_ps = ps2.tile([P, cfg.G], bf16, tag="tps")
                    nc.tensor.transpose(pT_ps[0:HID, 0:cfg.G], plb[:], ident[0:cfg.G, 0:cfg.G])
                    pT = sb.tile([P, cfg.G], bf16, tag="pT")
                    nc.vector.tensor_copy(pT[0:HID, :], pT_ps[0:HID, :])
                    wl_sb = sb.tile([P, cfg.OUT], bf16, tag="wlsb")
                    nc.sync.dma_start(wl_sb[0:HID, :], wlin[:])
                    fin_ps = ps.tile([cfg.G, cfg.OUT], f32, tag="gemm")
                    nc.tensor.matmul(fin_ps[:], pT[0:HID, 0:cfg.G],
                                     wl_sb[0:HID, :], start=True, stop=True)
                    bl_sb = sb.tile([1, cfg.OUT], f32, tag="blsb")
                    nc.sync.dma_start(bl_sb[:], blinrow[:])
                    blb = sb.tile([1, cfg.OUT], bf16, tag="blb")
                    nc.vector.tensor_copy(blb[:], bl_sb[:])
                    blbc_ps = ps2.tile([cfg.G, cfg.OUT], f32, tag="edps")
                    nc.tensor.matmul(blbc_ps[:], onesG[:], blb[:],
                                     start=True, stop=True)
                    fin_sb = sb.tile([cfg.G, cfg.OUT], f32, tag="finsb")
                    nc.vector.tensor_copy(fin_sb[:], blbc_ps[:])
                    nc.vector.tensor_tensor(out=fin_sb[:], in0=fin_ps[:],
                                            in1=fin_sb[:], op=mybir.AluOpType.add)
                    nc.sync.dma_start(out_t[:], fin_sb[:])
    nc.finalize()
    return nc


# ======================= harness entry =======================
_CACHE = {}


def _install_ntff_hook():
    try:
        import types
        if "antenv.axon_hooks" in sys.modules:
            return
        import antenv
        mod = types.ModuleType("antenv.axon_hooks")
        _state = {"hook": None}
        mod.set_axon_ntff_profile_hook = lambda h: _state.__setitem__("hook", h)
        mod.get_axon_ntff_profile_hook = lambda: _state["hook"]
        sys.modules["antenv.axon_hooks"] = mod
        antenv.axon_hooks = mod
        from trn_agent_boot.trn_boot import _ntff_profile_via_ctypes
        mod.set_axon_ntff_profile_hook(
            _ntff_profile_via_ctypes("/opt/axon/libaxon_pjrt.so"))
    except Exception:
        pass


def kernel(**inputs):
    import os
    from concourse import bass_utils
    x = np.asarray(inputs["x"], np.float32)
    ei = np.asarray(inputs["edge_index"], np.int64)
    batch = np.asarray(inputs["batch"], np.int64)
    cfg = Cfg(N=x.shape[0], E=ei.shape[1], G=64, IN=x.shape[1], HID=128,
              HEADS=2, OUT=64, ncores=8)
    key = (ei.tobytes(), batch.tobytes())
    kh = hash(key)
    if kh in _CACHE:
        per_core, meta, nc = _CACHE[kh]
    else:
        per_core, meta = host_prep(cfg, ei, batch)
        nc = build_kernel(cfg, meta)
        _CACHE[kh] = (per_core, meta, nc)
    wts = build_weights(
        inputs["W1"], inputs["as1"], inputs["ad1"], inputs["W2"],
        inputs["as2"], inputs["ad2"], inputs["W3"], inputs["as3"],
        inputs["ad3"], inputs["Wlin"], inputs["b3"], inputs["blin"],
        inputs["g1"], inputs["be1"], inputs["g2"], inputs["be2"])
    in_maps = []
    for c in range(cfg.ncores):
        d = {k: v for k, v in per_core[c].items() if k != "order"}
        d.update(wts)
        order = per_core[c]["order"]
        xs = np.zeros((cfg.npc_pad, cfg.IN), np.float32)
        valid = order >= 0
        xs[valid] = x[order[valid]]
        d["xT"] = np.ascontiguousarray(xs.T).astype(ml_dtypes.bfloat16)
        in_maps.append(d)
    trace = os.environ.get("GAT_TRACE", "0") == "1"
    if trace:
        _install_ntff_hook()
    res = bass_utils.run_bass_kernel_spmd(
        nc, in_maps, core_ids=list(range(cfg.ncores)), trace=trace)
    if trace and res.exec_time_ns is not None:
        print(f"HW exec time: {res.exec_time_ns} ns")
        kernel.last_exec_time_ns = res.exec_time_ns
        kernel.last_scope_times = res.per_core_scope_times
        kernel.last_trace = res.instructions_and_trace
    return np.asarray(res.results[0]["out"], np.float32)
